# revision 4
# baseline (speedup 1.0000x reference)
"""Trainium2 Bass kernel for conv-QK causal attention + MLP.

Reference computation (B=4, T=2048, D=512, H=8, DK=DV=64, FS=3):
  q = causal_conv1d(x, Wq) + bq ; k = causal_conv1d(x, Wk) + bk
  v = x @ Wv + bv
  per-head causal attention (softmax(q k^T / 8))
  out = relu(attn @ W1 + b1) @ W2 + b2        -> [B, T, 64]

Sharding: head-parallel, one head per NeuronCore (H == 8 == n_cores).
Each core computes q/k/v and attention for its head over all batches,
its partial attn @ W1[head], then a ReduceScatter sums the head partials
and shards tokens 8-way for the final relu/W2 epilogue.

On-chip layout is "transposed" (channels on partitions, tokens on the
free axis) so softmax needs no transposes at all:
  St[k, q] = K Q^T computed per 128-row k-block strip; exp on ScalarE
  (no max subtraction: logits are O(1) by construction); causal masking
  via a 0/1 upper-triangular multiply on the diagonal block; P V done as
  O~^T = [V | 1]^T P^T which also accumulates the softmax denominators
  as row 64 of the PSUM accumulator.
"""

import numpy as np

import concourse.bass as bass
import concourse.mybir as mybir
import concourse.tile as tile
from concourse import bacc, bass_utils
from concourse.masks import make_identity, make_upper_triangular

B, T, D = 4, 2048, 512
H, DK, DV, FS = 8, 64, 64, 3
NCORES = 8
TP = T + FS - 1          # left-zero-padded time axis (2050)
NDT = D // 128           # d-tiles (4)
NTT = T // 128           # t-tiles (16)
TSH = T * NCORES // NCORES  # tokens per core after reduce-scatter: B*T/8
TOK = B * T // NCORES    # 1024 output tokens per core

F32 = mybir.dt.float32
F32R = mybir.dt.float32r

_STATE = {}


def _build():
    nc = bacc.Bacc("TRN2", target_bir_lowering=False, debug=False,
                   num_devices=NCORES)

    xtp = nc.dram_tensor("xtp", [B, D, TP], F32R, kind="ExternalInput")
    wqk = nc.dram_tensor("wqk", [FS, NDT, 128, 128], F32R, kind="ExternalInput")
    wv = nc.dram_tensor("wv", [NDT, 128, DV], F32R, kind="ExternalInput")
    w1 = nc.dram_tensor("w1", [DV, 64], F32R, kind="ExternalInput")
    w2 = nc.dram_tensor("w2", [64, 64], F32R, kind="ExternalInput")
    bqk = nc.dram_tensor("bqk", [128, 1], F32, kind="ExternalInput")
    bv = nc.dram_tensor("bv", [64, 1], F32, kind="ExternalInput")
    b1 = nc.dram_tensor("b1", [64, 1], F32, kind="ExternalInput")
    b2 = nc.dram_tensor("b2", [64, 1], F32, kind="ExternalInput")
    out = nc.dram_tensor("out", [TOK, DV], F32, kind="ExternalOutput")

    with tile.TileContext(nc) as tc:
        with (
            tc.tile_pool(name="cpool", bufs=1) as cpool,
            tc.tile_pool(name="xpool", bufs=2) as xpool,
            tc.tile_pool(name="qkpool", bufs=2) as qkpool,
            tc.tile_pool(name="vtpool", bufs=1) as vtpool,
            tc.tile_pool(name="vpool", bufs=2) as vpool,
            tc.tile_pool(name="ptpool", bufs=3) as ptpool,
            tc.tile_pool(name="atpool", bufs=1) as atpool,
            tc.tile_pool(name="spool", bufs=1) as spool,
            tc.tile_pool(name="mpool", bufs=2, space="PSUM") as mpool,
            tc.tile_pool(name="opool", bufs=1, space="PSUM") as opool,
            tc.tile_pool(name="dpool", bufs=1, space="DRAM") as dpool,
        ):
            # ---- constants ----
            wqk_sb = cpool.tile([128, FS, NDT, 128], F32R)
            nc.sync.dma_start(wqk_sb[:], wqk.ap().rearrange("f dt p m -> p f dt m"))
            wv_sb = cpool.tile([128, NDT, DV], F32R)
            nc.sync.dma_start(wv_sb[:], wv.ap().rearrange("dt p m -> p dt m"))
            w1_sb = cpool.tile([DV, 64], F32R)
            nc.sync.dma_start(w1_sb[:], w1.ap())
            w2_sb = cpool.tile([64, 64], F32R)
            nc.sync.dma_start(w2_sb[:], w2.ap())
            bqk_sb = cpool.tile([128, 1], F32)
            nc.sync.dma_start(bqk_sb[:], bqk.ap())
            bv_sb = cpool.tile([64, 1], F32)
            nc.sync.dma_start(bv_sb[:], bv.ap())
            b1_sb = cpool.tile([64, 1], F32)
            nc.sync.dma_start(b1_sb[:], b1.ap())
            b2_sb = cpool.tile([64, 1], F32)
            nc.sync.dma_start(b2_sb[:], b2.ap())
            mask_sb = cpool.tile([128, 128], F32)
            make_upper_triangular(nc, mask_sb[:], val=1.0, diag=True)
            ident_sb = cpool.tile([64, 64], F32)
            make_identity(nc, ident_sb[:])

            rs_in = dpool.tile([NCORES, 64, TOK], F32)
            rs_out = dpool.tile([64, TOK], F32)

            EXP = mybir.ActivationFunctionType.Exp

            for b in range(B):
                xtp_sb = xpool.tile([128, NDT, TP], F32R)
                nc.sync.dma_start(
                    xtp_sb[:], xtp.ap()[b].rearrange("(dt p) t -> p dt t", p=128))

                # ---- QK projection: rows 0:64 = Q^T, rows 64:128 = K^T ----
                qkt_sb = qkpool.tile([128, T], F32R, tag="qkt")
                qk2_sb = qkpool.tile([128, T], F32R, tag="qk2")  # swapped halves
                for tci in range(4):
                    sl = bass.ts(tci, 512)
                    qk_ps = mpool.tile([128, 512], F32, tag="ps_small")
                    n_mm = FS * NDT
                    i = 0
                    for f in range(FS):
                        for dt_ in range(NDT):
                            nc.tensor.matmul(
                                qk_ps[:],
                                wqk_sb[:, f, dt_, :],
                                xtp_sb[:, dt_, tci * 512 + f: tci * 512 + f + 512],
                                start=(i == 0), stop=(i == n_mm - 1))
                            i += 1
                    nc.vector.tensor_scalar_add(qkt_sb[:, sl], qk_ps[:], bqk_sb[:])
                    # replicas on the opposite partition halves (for 2x row
                    # packing of the K=64 attention matmuls)
                    nc.sync.dma_start(qk2_sb[64:128, sl], qkt_sb[0:64, sl])
                    nc.sync.dma_start(qk2_sb[0:64, sl], qkt_sb[64:128, sl])

                # ---- V^T then transpose to [t, 65] with trailing ones ----
                vt_sb = vtpool.tile([64, T], F32)
                for tci in range(4):
                    sl = bass.ts(tci, 512)
                    vt_ps = mpool.tile([64, 512], F32, tag="ps_small")
                    for dt_ in range(NDT):
                        nc.tensor.matmul(
                            vt_ps[:],
                            wv_sb[:, dt_, :],
                            xtp_sb[:, dt_, tci * 512 + 2: tci * 512 + 2 + 512],
                            start=(dt_ == 0), stop=(dt_ == NDT - 1))
                    nc.vector.tensor_scalar_add(vt_sb[:, sl], vt_ps[:], bv_sb[:])
                v_sb = vpool.tile([128, NTT, DV + 1], F32R)
                nc.gpsimd.memset(v_sb[:, :, DV:DV + 1].bitcast(F32), 1.0)
                for tt in range(NTT):
                    tr_ps = mpool.tile([128, 64], F32, tag="ps_small")
                    nc.tensor.transpose(
                        tr_ps[:], vt_sb[:, bass.ts(tt, 128)], ident_sb[:])
                    nc.vector.tensor_copy(v_sb[:, tt, 0:DV], tr_ps[:])

                # ---- attention, two q-passes of 1024 columns ----
                attnT_sb = atpool.tile([64, T], F32R, tag="attnT")
                for ps in range(2):
                    qlo, qhi = ps * 1024, ps * 1024 + 1024
                    o_ps = opool.tile([DV + 1, 1024], F32)
                    nkb = (qhi // 128)
                    for kba in range(0, nkb, 2):
                        strips = []
                        for kb in (kba, kba + 1):
                            qs = max(qlo, kb * 128)
                            w = qhi - qs
                            pt = ptpool.tile([128, 1024], F32R, tag="pt")
                            st_ps = mpool.tile([128, 1024], F32, tag="st")
                            if kb % 2 == 0:
                                lhsT = qk2_sb[0:64, bass.ts(kb, 128)]
                            else:
                                lhsT = qkt_sb[64:128, bass.ts(kb, 128)]
                            for c0 in range(0, w, 512):
                                cw = min(512, w - c0)
                                if kb % 2 == 0:
                                    rhs = qkt_sb[0:64, qs + c0: qs + c0 + cw]
                                else:
                                    rhs = qk2_sb[64:128, qs + c0: qs + c0 + cw]
                                nc.tensor.matmul(
                                    st_ps[:, c0:c0 + cw], lhsT, rhs,
                                    start=True, stop=True)
                            nc.scalar.activation(
                                pt[:, 0:w], st_ps[:, 0:w], EXP, scale=0.125)
                            if kb * 128 >= qlo:
                                nc.vector.tensor_mul(
                                    pt[:, 0:128], pt[:, 0:128], mask_sb[:])
                            strips.append((kb, qs, w, pt))
                        for kb, qs, w, pt in strips:
                            for qc in range(2):
                                clo = qlo + qc * 512
                                chi = clo + 512
                                lo = max(clo, qs)
                                n = chi - lo
                                if n <= 0:
                                    continue
                                nc.tensor.matmul(
                                    o_ps[:, lo - qlo: lo - qlo + n],
                                    v_sb[:, kb, :],
                                    pt[:, lo - qs: lo - qs + n],
                                    start=(kb == 0),
                                    stop=(kb == chi // 128 - 1))
                    # normalize: attnT = O~[0:64] / l,  l = O~[64]
                    linv = spool.tile([1, 1024], F32, tag="linv")
                    nc.vector.reciprocal(linv[:], o_ps[DV:DV + 1, :])
                    linv_bc = spool.tile([64, 1024], F32, tag="linv_bc")
                    nc.gpsimd.partition_broadcast(linv_bc[:], linv[:])
                    nc.vector.tensor_mul(
                        attnT_sb[:, qlo:qhi], o_ps[0:DV, :], linv_bc[:])

                # ---- head-partial of the first MLP matmul ----
                p1_sb = atpool.tile([64, T], F32, tag="p1")
                for tci in range(4):
                    sl = bass.ts(tci, 512)
                    p1_ps = mpool.tile([64, 512], F32, tag="ps_small")
                    nc.tensor.matmul(p1_ps[:], w1_sb[:], attnT_sb[:, sl],
                                     start=True, stop=True)
                    nc.vector.tensor_copy(p1_sb[:, sl], p1_ps[:])
                nc.sync.dma_start(rs_in[2 * b], p1_sb[:, 0:1024])
                nc.sync.dma_start(rs_in[2 * b + 1], p1_sb[:, 1024:2048])

            # ---- sum over heads, shard tokens 8-way ----
            nc.gpsimd.collective_compute(
                "ReduceScatter",
                mybir.AluOpType.add,
                replica_groups=[list(range(NCORES))],
                ins=[rs_in.opt()],
                outs=[rs_out.opt()],
            )

            # ---- epilogue: relu(z + b1) @ W2 + b2, untranspose, store ----
            zin_sb = spool.tile([64, TOK], F32, tag="zin")
            nc.sync.dma_start(zin_sb[:], rs_out[:])
            z_sb = spool.tile([64, TOK], F32R, tag="z")
            nc.vector.tensor_scalar(
                z_sb[:], zin_sb[:], b1_sb[:], 0.0,
                op0=mybir.AluOpType.add, op1=mybir.AluOpType.max)
            fin_sb = spool.tile([64, TOK], F32, tag="fin")
            for fc in range(2):
                sl = bass.ts(fc, 512)
                f_ps = mpool.tile([64, 512], F32, tag="ps_small")
                nc.tensor.matmul(f_ps[:], w2_sb[:], z_sb[:, sl],
                                 start=True, stop=True)
                nc.vector.tensor_scalar_add(fin_sb[:, sl], f_ps[:], b2_sb[:])
            out_sb = spool.tile([128, TOK // 128, DV], F32, tag="outsb")
            for tt in range(TOK // 128):
                tr_ps = mpool.tile([128, 64], F32, tag="ps_small")
                nc.tensor.transpose(
                    tr_ps[:], fin_sb[:, bass.ts(tt, 128)], ident_sb[:])
                nc.vector.tensor_copy(out_sb[:, tt, :], tr_ps[:])
            nc.sync.dma_start(
                out.ap().rearrange("(tt p) e -> p tt e", p=128), out_sb[:])

    nc.compile()
    return nc


def _get_nc():
    if "nc" not in _STATE:
        _STATE["nc"] = _build()
    return _STATE["nc"]


def _prep_inputs(x, Wq, bq, Wk, bk, Wv, bv, W1, b1, W2, b2):
    f = np.float32
    x = np.ascontiguousarray(np.asarray(x, f))
    xtp = np.zeros((B, D, TP), f)
    xtp[:, :, FS - 1:] = x.transpose(0, 2, 1)
    Wq = np.asarray(Wq, f)
    Wk = np.asarray(Wk, f)
    Wv = np.asarray(Wv, f)
    W1 = np.asarray(W1, f)
    W2 = np.asarray(W2, f)
    bq = np.asarray(bq, f)
    bk = np.asarray(bk, f)
    bv = np.asarray(bv, f)
    b1 = np.asarray(b1, f)
    b2 = np.asarray(b2, f)

    in_maps = []
    for c in range(NCORES):
        hs = slice(c * DK, (c + 1) * DK)
        wqk_c = np.concatenate([Wq[:, :, hs], Wk[:, :, hs]], axis=2)
        in_maps.append({
            "xtp": xtp,
            "wqk": np.ascontiguousarray(
                wqk_c.reshape(FS, NDT, 128, 128)),
            "wv": np.ascontiguousarray(Wv[:, hs].reshape(NDT, 128, DV)),
            "w1": np.ascontiguousarray(W1[hs, :]),
            "w2": np.ascontiguousarray(W2),
            "bqk": np.ascontiguousarray(
                np.concatenate([bq[hs], bk[hs]])[:, None]),
            "bv": np.ascontiguousarray(bv[hs][:, None]),
            "b1": np.ascontiguousarray(b1[:, None]),
            "b2": np.ascontiguousarray(b2[:, None]),
        })
    return in_maps


def _run(inputs, trace=False):
    nc = _get_nc()
    in_maps = _prep_inputs(**inputs)
    r = bass_utils.run_bass_kernel_spmd(
        nc, in_maps, core_ids=list(range(NCORES)), trace=trace)
    pieces = [r.results[c]["out"] for c in range(NCORES)]
    full = np.concatenate(pieces, axis=0).reshape(B, T, DV)
    return full, r


def kernel(**inputs):
    full, _ = _run(inputs, trace=False)
    return full
